# revision 20
# baseline (speedup 1.0000x reference)
"""GCNAggregator Trainium2 Bass kernel (v2: bf16 gather, folded self-loop).

out[i] = (sum_{e: seg[e]==i} features[neighbor_idx[e]] + features[i]) / (deg_i + 1)

Strategy (8 NeuronCores, SPMD):
  - Nodes are sharded into 8 contiguous, edge-balanced ranges (<=6272 nodes
    each). Since segment_ids is sorted, each core's incident edges are a
    contiguous range of the edge list. A self-edge (i -> i) is appended for
    every node, so the self term and the +1 denominator fold into the
    ordinary segment sum.
  - Per core, nodes are packed greedily into "slots" of <=128 consecutive
    nodes, capped by per-slot edge counts so every slot is nearly full of
    edges. Slot tile counts are maxed over the 8 cores so the compiled
    program is identical on every core (SPMD) at ~2% gather padding.
  - Features are stored as a bf16 table (512B/row: the DMA-efficiency knee;
    fp32 rows would double the dominant gather traffic for precision far
    beyond the 2e-2 requirement) and each slot's edges are gathered from
    HBM with gpsimd.dma_gather, then segment-summed on the tensor engine
    via one 256-wide one-hot matmul per 128-edge tile:
        psum[128 nodes, 256] += onehot[128 edges, 128 nodes]^T @ G[128, 256]
    The one-hot is exact in bf16 and PSUM accumulates in fp32, so the only
    losses are the bf16 feature quantization (~2^-9 relative) and the bf16
    output store; end-to-end rel err ~1e-3.
  - dma_gather indices are int16 (max 32767) but the table has 50000 rows,
    so each slot's edges are split into a low class (row < 32768) and a
    high class (row >= 32768, gathered from an offset view of the table).
  - Finalize per slot: out = psum * 1/(deg+1) (one DVE op), DMA out (bf16;
    the host upcasts to fp32).

The host only computes integer index metadata (shard boundaries, per-slot
class-split index streams, relative segment ids, degrees) plus the bf16
downcast of the table; all arithmetic (gather, segment sum, normalize)
runs on device.
"""

import sys

import numpy as np

try:
    import concourse  # noqa: F401
except ImportError:  # pragma: no cover
    sys.path.insert(0, "/opt/trn_rl_repo")

from contextlib import ExitStack

import concourse.mybir as mybir
from concourse import bacc, bass_utils, tile

N_NODES = 50000
N_EDGES = 1_600_000
D = 256
N_CORES = 8
NPC = 6272          # node slots per core (= 49 * 128)
SPLIT = 32768       # int16 gather-index window

_PROGRAM_CACHE: dict = {}
LAST_NC = None  # exposed for test harness introspection (TimelineSim)

MAX_GATHER = 1024   # rows per dma_gather call (SWDGE ring capacity)
DMA_SCRATCH = 16384  # bytes/partition of SWDGE descriptor ring (default)
import os as _os
GROUP = int(_os.environ.get("GROUP", "2"))  # slots per gather group (amortizes SWDGE overhead)


def _groups(n_slots):
    return [
        list(range(i, min(i + GROUP, n_slots))) for i in range(0, n_slots, GROUP)
    ]


def _tile_layout(t_l_arr, t_h_arr):
    """Global tile order: per group, the L tiles of its slots then the H
    tiles of its slots (so each class is one contiguous gather stream per
    group). Returns (lbase, hbase, group_off, group_L, group_H)."""
    n_slots = len(t_l_arr)
    lbase = [0] * n_slots
    hbase = [0] * n_slots
    group_off = []
    group_L = []
    group_H = []
    off = 0
    for gr in _groups(n_slots):
        group_off.append(off)
        L = sum(t_l_arr[s] for s in gr)
        H = sum(t_h_arr[s] for s in gr)
        lo = off
        ho = off + L
        for s in gr:
            lbase[s] = lo
            hbase[s] = ho
            lo += t_l_arr[s]
            ho += t_h_arr[s]
        group_L.append(L)
        group_H.append(H)
        off += L + H
    return lbase, hbase, group_off, group_L, group_H


def _chunks(total_rows):
    out = []
    off = 0
    while off < total_rows:
        k = min(MAX_GATHER, total_rows - off)
        out.append((off, k))
        off += k
    return out


def _m_dense_off(m_l_arr, m_h_arr):
    off = [0] * len(m_l_arr)
    acc = 0
    for g in range(len(m_l_arr)):
        off[g] = acc
        acc += m_l_arr[g] + m_h_arr[g]
    return off


def _build_program(
    t_l_arr: tuple, t_h_arr: tuple, m_l_arr: tuple, m_h_arr: tuple, w_arr: tuple
):
    """Build + compile the (uniform-across-cores, SPMD) per-core program.

    t_l_arr/t_h_arr: per-slot tile counts (max over the 8 cores), so the
    program structure is identical on every core while padding stays low.
    """
    import os

    n_slots = len(t_l_arr)
    tiles_g = [t_l_arr[g] + t_h_arr[g] for g in range(n_slots)]
    nt_tot = sum(tiles_g)
    rows_tot = nt_tot * 128
    ni16 = rows_tot // 16  # gidx columns (wrapped-16 int16 layout)
    nc = bacc.Bacc(
        "TRN2", target_bir_lowering=False, debug=False, num_devices=N_CORES,
        dynamic_dma_scratch_size=DMA_SCRATCH,
    )

    feat_d = nc.dram_tensor(
        "featbf", (N_NODES, D), mybir.dt.bfloat16, kind="ExternalInput"
    ).ap()
    gidx_d = nc.dram_tensor(
        "gidx", (32, ni16), mybir.dt.int16, kind="ExternalInput"
    ).ap()
    srel_d = nc.dram_tensor(
        "srel", (128, nt_tot), mybir.dt.int8, kind="ExternalInput"
    ).ap()
    nm_tot = max(sum(m_l_arr) + sum(m_h_arr), 1)
    srelb_d = nc.dram_tensor(
        "srelb", (128, nm_tot), mybir.dt.int8, kind="ExternalInput"
    ).ap()
    cnt1_d = nc.dram_tensor(
        "cnt1", (128, n_slots), mybir.dt.float32, kind="ExternalInput"
    ).ap()
    out_d = nc.dram_tensor(
        "out", (n_slots * 128, D), mybir.dt.bfloat16, kind="ExternalOutput"
    ).ap()

    m_dense_off = _m_dense_off(m_l_arr, m_h_arr)
    feat_lo = feat_d[0:SPLIT, :]
    feat_hi = feat_d[N_NODES - SPLIT : N_NODES, :]

    with tile.TileContext(nc) as tc:
        with ExitStack() as ctx:
            gb = int(os.environ.get("GT_BUFS", "3"))
            ob = int(os.environ.get("OH_BUFS", "24"))
            fb = int(os.environ.get("FIN_BUFS", "3"))
            pb = int(os.environ.get("PSUM_BUFS", "5"))
            const_pool = ctx.enter_context(tc.tile_pool(name="const", bufs=1))
            g_pool = ctx.enter_context(tc.tile_pool(name="gt", bufs=gb))
            oh_pool = ctx.enter_context(tc.tile_pool(name="oh", bufs=ob))
            fin_pool = ctx.enter_context(tc.tile_pool(name="fin", bufs=fb))
            psum_pool = ctx.enter_context(
                tc.tile_pool(name="psum", bufs=pb, space="PSUM")
            )

            # persistent metadata in SBUF. gidx arrives as two 16-partition
            # wraps (1/4 of the SWDGE index format, which wants the wrap
            # replicated on all 128 partitions) and is replicated on-device
            # by two quadrant-aligned DVE partition-doubling copies. Both
            # gidx and srel are loaded lazily per group (a couple of groups
            # ahead) so the first gathers aren't queued behind the full
            # metadata transfer on the serial DMA resource.
            gidx_sb = const_pool.tile([128, ni16], mybir.dt.int16)
            srel_i8 = const_pool.tile([128, nt_tot], mybir.dt.int8)
            srel_sb = const_pool.tile([128, nt_tot], mybir.dt.float32)
            srelb_i8 = const_pool.tile([128, nm_tot], mybir.dt.int8)
            srelb_sb = const_pool.tile([128, nm_tot], mybir.dt.float32)
            cnt1_sb = const_pool.tile([128, n_slots], mybir.dt.float32)
            # group 0's gidx first so its gathers start immediately; the
            # rest of gidx, then srel/cnt1, follow on the serial DMA
            # resource while the first gathers' descriptors are generated
            c_split = min(
                (t_l_arr[0] + t_h_arr[0] + t_l_arr[1] + t_h_arr[1]) * 8, ni16
            )
            nc.sync.dma_start(gidx_sb[0:32, 0:c_split], gidx_d[:, 0:c_split])
            for p in (32, 64):
                nc.vector.tensor_copy(
                    gidx_sb[p : 2 * p, 0:c_split], gidx_sb[0:p, 0:c_split]
                )
            nc.sync.dma_start(
                gidx_sb[0:32, c_split:ni16], gidx_d[:, c_split:ni16]
            )
            for p in (32, 64):
                nc.vector.tensor_copy(
                    gidx_sb[p : 2 * p, c_split:ni16],
                    gidx_sb[0:p, c_split:ni16],
                )
            t_split = c_split // 8
            nc.sync.dma_start(srel_i8[:, 0:t_split], srel_d[:, 0:t_split])
            nc.vector.tensor_copy(srel_sb[:, 0:t_split], srel_i8[:, 0:t_split])
            nc.sync.dma_start(
                srel_i8[:, t_split:nt_tot], srel_d[:, t_split:nt_tot]
            )
            nc.vector.tensor_copy(
                srel_sb[:, t_split:nt_tot], srel_i8[:, t_split:nt_tot]
            )
            # srelB packs only the multi-tile columns (~7% of tiles)
            nc.sync.dma_start(srelb_i8[:], srelb_d[:])
            nc.vector.tensor_copy(srelb_sb[:], srelb_i8[:])
            nc.sync.dma_start(cnt1_sb[:], cnt1_d[:])

            iota_i = const_pool.tile([128, 128], mybir.dt.int32)
            nc.gpsimd.iota(iota_i[:], pattern=[[1, 128]], base=0, channel_multiplier=0)
            iota_f = const_pool.tile([128, 128], mybir.dt.bfloat16)
            nc.vector.tensor_copy(iota_f[:], iota_i[:])

            lbase, hbase, group_off, group_L, group_H = _tile_layout(
                t_l_arr, t_h_arr
            )
            max_gtiles = max(
                L + H for L, H in zip(group_L, group_H)
            )
            for gi, gr in enumerate(_groups(n_slots)):
                m0 = int(group_off[gi])
                L = group_L[gi]
                c0 = m0 * 8  # 128 rows -> 8 int16-wrapped columns
                gt = g_pool.tile(
                    [128, max_gtiles, D], mybir.dt.bfloat16, tag="gt"
                )
                for off, k in _chunks(L * 128):
                    nc.gpsimd.dma_gather(
                        gt[:, off // 128 : (off + k) // 128, :], feat_lo,
                        gidx_sb[:, c0 + off // 16 : c0 + (off + k) // 16],
                        num_idxs=k, num_idxs_reg=k,
                        elem_size=D, elem_step=D,
                    )
                for off, k in _chunks(group_H[gi] * 128):
                    nc.gpsimd.dma_gather(
                        gt[:, L + off // 128 : L + (off + k) // 128, :],
                        feat_hi,
                        gidx_sb[
                            :,
                            c0 + L * 8 + off // 16 : c0 + L * 8 + (off + k) // 16,
                        ],
                        num_idxs=k, num_idxs_reg=k,
                        elem_size=D, elem_step=D,
                    )

                for g in gr:
                    psum = psum_pool.tile([128, D], mybir.dt.float32, tag="ps")
                    mb = m_dense_off[g]
                    tiles_s = [
                        (lbase[g] - m0 + t, t if t < m_l_arr[g] else -1)
                        for t in range(t_l_arr[g])
                    ] + [
                        (
                            hbase[g] - m0 + t,
                            m_l_arr[g] + t if t < m_h_arr[g] else -1,
                        )
                        for t in range(t_h_arr[g])
                    ]
                    for i, (t, mcol) in enumerate(tiles_s):
                        is_multi = mcol >= 0
                        oh = oh_pool.tile([128, 128], mybir.dt.bfloat16, tag="oh")
                        nc.vector.tensor_scalar(
                            oh[:], iota_f[:],
                            srel_sb[:, m0 + t : m0 + t + 1], None,
                            op0=mybir.AluOpType.is_equal,
                        )
                        if is_multi:
                            # leading tiles hold the slot's multi-target
                            # rows: add the second-target one-hot so one
                            # gathered row feeds two destination nodes
                            oh2 = oh_pool.tile(
                                [128, 128], mybir.dt.bfloat16, tag="oh2"
                            )
                            nc.vector.tensor_scalar(
                                oh2[:], iota_f[:],
                                srelb_sb[:, mb + mcol : mb + mcol + 1], None,
                                op0=mybir.AluOpType.is_equal,
                            )
                            nc.vector.tensor_add(oh[:], oh[:], oh2[:])
                        nc.tensor.matmul(
                            psum[:], oh[:], gt[:, t, :],
                            start=(i == 0), stop=(i == len(tiles_s) - 1),
                        )

                    rec = fin_pool.tile([128, 1], mybir.dt.float32, tag="rec")
                    nc.vector.reciprocal(rec[:], cnt1_sb[:, g : g + 1])
                    o_sb = fin_pool.tile([128, D], mybir.dt.bfloat16, tag="o")
                    w = w_arr[g]
                    nc.vector.tensor_scalar_mul(
                        o_sb[0:w, :], psum[0:w, :], rec[0:w, :]
                    )
                    nc.sync.dma_start(
                        out_d[g * 128 : g * 128 + w, :], o_sb[0:w, :]
                    )

    nc.compile()
    return nc


OV_LO = N_NODES - SPLIT  # B-view base: view A = rows [0, SPLIT), B = [OV_LO, N)
# rows in [OV_LO, SPLIT) are reachable from BOTH views: per slot they are
# assigned to whichever class makes the A-run an exact multiple of 128 rows
# (aligned across all 8 cores), so class-rounding waste only hits the B run.


def _pack_slots_tot(cum_t, n_nodes, cap):
    """Greedy variable-width node slots: each slot takes consecutive nodes
    (<=128) while its TOTAL edge count stays under cap*128. Returns a list
    of (base, width)."""
    slots = []
    i = 0
    while i < n_nodes:
        jmax = min(i + 128, n_nodes)
        jt = int(np.searchsorted(cum_t, cum_t[i] + cap * 128, side="right")) - 1
        j = max(min(jmax, jt), i + 1)
        slots.append((i, j - i))
        i = j
    return slots


def _preprocess(features, neighbor_idx, segment_ids):
    """Host-side shard/index metadata construction (integers only, plus the
    bf16 downcast of the feature table)."""
    feat = np.ascontiguousarray(np.asarray(features, dtype=np.float32))
    seg = np.asarray(segment_ids).astype(np.int64)
    nid = np.asarray(neighbor_idx).astype(np.int64)
    n_edges = seg.shape[0]

    bf16 = mybir.dt.np(mybir.dt.bfloat16)
    featbf = np.ascontiguousarray(feat.astype(bf16))

    # edge-balanced core node boundaries (spans capped at NPC node slots)
    bounds = [0]
    for c in range(1, N_CORES):
        n = int(seg[min(c * n_edges // N_CORES, n_edges - 1)])
        n = min(n, bounds[-1] + NPC)
        n = max(n, N_NODES - (N_CORES - c) * NPC, bounds[-1])
        bounds.append(n)
    bounds.append(N_NODES)

    # per-core edge slices (self-edges folded in) and class prefix sums
    per_core = []
    for c in range(N_CORES):
        lo, hi = np.searchsorted(seg, [bounds[c], bounds[c + 1]])
        nn = bounds[c + 1] - bounds[c]
        s = np.concatenate([seg[lo:hi] - bounds[c], np.arange(nn)])
        x = np.concatenate([nid[lo:hi], np.arange(bounds[c], bounds[c + 1])])
        o = np.argsort(s, kind="stable")
        s = s[o]
        x = x[o]
        cnt_a = np.bincount(s[x < OV_LO], minlength=nn)       # A-only
        cnt_o = np.bincount(
            s[(x >= OV_LO) & (x < SPLIT)], minlength=nn
        )                                                     # movable
        cnt_t = np.bincount(s, minlength=nn)
        cum_a = np.concatenate([[0], np.cumsum(cnt_a)])
        cum_o = np.concatenate([[0], np.cumsum(cnt_o)])
        cum_t = np.concatenate([[0], np.cumsum(cnt_t)])
        per_core.append((s, x, nn, cum_a, cum_o, cum_t))

    # choose the total-edge cap minimizing the shared tile count
    # score each cap with the exact deduped per-slot class counts the tile
    # layout will actually use (raw-count scoring picks a slightly wrong cap)
    def _dedup_counts(all_slots):
        ns = max(len(sl) for sl in all_slots)
        n_cg = np.zeros((N_CORES, ns), np.int64)
        a_cg = np.zeros((N_CORES, ns), np.int64)
        o_cg = np.zeros((N_CORES, ns), np.int64)
        for c in range(N_CORES):
            s_, x_, nn = per_core[c][:3]
            slots = all_slots[c]
            node_bnds = [sl[0] for sl in slots] + [nn]
            edge_bnds = np.searchsorted(s_, node_bnds)
            for g in range(len(slots)):
                a, b = edge_bnds[g], edge_bnds[g + 1]
                xs = np.sort(x_[a:b], kind="stable")
                n = len(xs)
                if n == 0:
                    continue
                new_run = np.empty(n, bool)
                new_run[0] = True
                np.not_equal(xs[1:], xs[:-1], out=new_run[1:])
                run_start = np.flatnonzero(new_run)
                run_id = np.cumsum(new_run) - 1
                r = np.arange(n) - run_start[run_id]
                xrow = xs[(r & 1) == 0]
                n_cg[c, g] = len(xrow)
                a_cg[c, g] = int(np.searchsorted(xrow, OV_LO))
                o_cg[c, g] = int(np.searchsorted(xrow, SPLIT)) - a_cg[c, g]
        return ns, n_cg, a_cg, o_cg

    def _tiles_for(ns, n_cg, a_cg, o_cg):
        lo_t = -(-a_cg // 128).max(axis=0)
        hi_t = np.maximum(((a_cg + o_cg) // 128).min(axis=0), lo_t)
        t_l = lo_t.copy()
        t_h = np.zeros_like(t_l)
        for g in range(ns):
            best_g = None
            for T in range(int(lo_t[g]), int(hi_t[g]) + 1):
                ac = np.minimum(128 * T, a_cg[:, g] + o_cg[:, g])
                th = int((-(-(n_cg[:, g] - ac) // 128)).max())
                if best_g is None or T + th < best_g[0]:
                    best_g = (T + th, T, th)
            t_l[g], t_h[g] = best_g[1], best_g[2]
        return t_l, t_h

    best = None
    for cap in range(24, 44):
        all_slots = [
            _pack_slots_tot(pc[5], pc[2], cap) for pc in per_core
        ]
        ns_c, n_c, a_c, o_c = _dedup_counts(all_slots)
        tl_c, th_c = _tiles_for(ns_c, n_c, a_c, o_c)
        total = int(tl_c.sum() + th_c.sum())
        if best is None or total < best[0]:
            best = (total, all_slots)
    _, all_slots = best
    ns = max(len(sl) for sl in all_slots)

    # dedup pass: within a slot, a source row referenced by several edges
    # is gathered once and carries up to two targets (sources with k
    # targets occupy ceil(k/2) rows). Multi-target rows are placed FIRST
    # within each class run so only the leading tile(s) of a run need the
    # two-pass multi-hot build on DVE; the rest keep the cheap one-hot.
    dedup = {}
    n_cg = np.zeros((N_CORES, ns), np.int64)
    a_cg = np.zeros((N_CORES, ns), np.int64)
    o_cg = np.zeros((N_CORES, ns), np.int64)
    for c in range(N_CORES):
        s, x, nn = per_core[c][:3]
        slots = all_slots[c]
        node_bnds = [sl[0] for sl in slots] + [nn]
        edge_bnds = np.searchsorted(s, node_bnds)
        for g, (base_n, width) in enumerate(slots):
            a, b = edge_bnds[g], edge_bnds[g + 1]
            xg = x[a:b]
            tg = s[a:b] - base_n
            o2 = np.argsort(xg, kind="stable")
            xs = xg[o2]
            ts = tg[o2]
            n = len(xs)
            if n == 0:
                dedup[(c, g)] = (xs, ts, ts)
                continue
            new_run = np.empty(n, bool)
            new_run[0] = True
            np.not_equal(xs[1:], xs[:-1], out=new_run[1:])
            run_start = np.flatnonzero(new_run)
            run_id = np.cumsum(new_run) - 1
            r = np.arange(n) - run_start[run_id]
            rows = np.flatnonzero((r & 1) == 0)
            nxt = rows + 1
            has_partner = nxt < n
            has_partner[has_partner] &= ~new_run[nxt[has_partner]]
            s_b = np.where(has_partner, ts[np.minimum(nxt, n - 1)], -1)
            xrow = xs[rows]
            dedup[(c, g)] = (xrow, ts[rows], s_b)
            n_cg[c, g] = len(xrow)
            a_cg[c, g] = int(np.searchsorted(xrow, OV_LO))
            o_cg[c, g] = int(np.searchsorted(xrow, SPLIT)) - a_cg[c, g]

    # exact per-slot A-run tile count on the deduped counts
    t_l, t_h = _tiles_for(ns, n_cg, a_cg, o_cg)
    t_l_arr = tuple(int(v) for v in t_l)
    t_h_arr = tuple(int(v) for v in t_h)
    # a slot with zero tiles would leave its PSUM accumulator unwritten
    t_l_arr = tuple(
        max(tl, 1) if tl + th == 0 else tl for tl, th in zip(t_l_arr, t_h_arr)
    )
    n_slots = len(t_l_arr)

    lbase, hbase, _, _, _ = _tile_layout(t_l_arr, t_h_arr)
    nt_tot = sum(t_l_arr) + sum(t_h_arr)

    # multi-tile counts per slot-class (max over cores), known before the
    # write pass: the class split at na = min(t_l*128, #A-capable rows)
    # and the multi-first reorder don't change the counts
    ml_cnt = np.zeros((N_CORES, n_slots), np.int64)
    mh_cnt = np.zeros((N_CORES, n_slots), np.int64)
    for c in range(N_CORES):
        for g in range(len(all_slots[c])):
            xrow, _, s_b = dedup[(c, g)]
            na = min(t_l_arr[g] * 128, int(np.searchsorted(xrow, SPLIT)))
            ml_cnt[c, g] = int((s_b[:na] >= 0).sum())
            mh_cnt[c, g] = int((s_b[na:] >= 0).sum())
    m_l_arr = tuple(
        min(int(-(-int(ml_cnt[:, g].max()) // 128)), t_l_arr[g])
        for g in range(n_slots)
    )
    m_h_arr = tuple(
        min(int(-(-int(mh_cnt[:, g].max()) // 128)), t_h_arr[g])
        for g in range(n_slots)
    )
    m_off_d = _m_dense_off(m_l_arr, m_h_arr)
    nm_tot = max(sum(m_l_arr) + sum(m_h_arr), 1)

    in_maps = []
    slot_maps = []
    for c in range(N_CORES):
        s, x, nn = per_core[c][:3]
        slots = all_slots[c]
        gidx_all = np.zeros(nt_tot * 128, np.int16)
        srel_all = np.full((nt_tot, 128), -1, np.int8)
        srelb_all = np.full((nm_tot, 128), -1, np.int8)
        cnt1 = np.ones((128, n_slots), np.float32)
        node_bnds = [sl[0] for sl in slots] + [nn]
        edge_bnds = np.searchsorted(s, node_bnds)
        for g, (base_n, width) in enumerate(slots):
            t_l, t_h = t_l_arr[g], t_h_arr[g]
            kl, kh = t_l * 128, t_h * 128
            a, b = edge_bnds[g], edge_bnds[g + 1]
            sg = s[a:b]
            # deduped source-sorted rows: class A = the first min(t_l*128,
            # #rows reachable from view A) rows, class B = the rest. Within
            # each class, multi-target rows go first so only the leading
            # tiles need the two-pass multi-hot build.
            xrow, s_a, s_b = dedup[(c, g)]
            n_av = int(np.searchsorted(xrow, SPLIT))
            na = min(kl, n_av)
            lb, hb = lbase[g] * 128, hbase[g] * 128
            for cls, (x0, sa0, sb0, base, tt, plane_off) in enumerate(
                (
                    (xrow[:na], s_a[:na], s_b[:na], lb, t_l, lbase[g]),
                    (
                        xrow[na:] - OV_LO,
                        s_a[na:],
                        s_b[na:],
                        hb,
                        t_h,
                        hbase[g],
                    ),
                )
            ):
                multi = sb0 >= 0
                order = np.argsort(~multi, kind="stable")
                x0 = x0[order]
                sa0 = sa0[order]
                sb0 = sb0[order]
                kk = tt * 128
                gidx_all[base : base + len(x0)] = x0.astype(np.int16)
                pl = np.full(kk, -1, np.int8)
                pl[: len(sa0)] = sa0
                srel_all[plane_off : plane_off + tt] = pl.reshape(tt, 128)
                mt = m_l_arr[g] if cls == 0 else m_h_arr[g]
                if mt:
                    mk = mt * 128
                    pb_ = np.full(mk, -1, np.int8)
                    nb = min(len(sb0), mk)
                    pb_[:nb] = sb0[:nb]
                    d0 = m_off_d[g] + (0 if cls == 0 else m_l_arr[g])
                    srelb_all[d0 : d0 + mt] = pb_.reshape(mt, 128)
            # count includes the folded-in self edge => deg + 1
            cnts = np.bincount(sg - base_n, minlength=width)[:width]
            cnt1[:width, g] = cnts
        gidx_w = np.ascontiguousarray(np.tile(gidx_all.reshape(-1, 16).T, (2, 1)))
        in_maps.append(
            {
                "featbf": featbf,
                "gidx": gidx_w,
                "srel": np.ascontiguousarray(srel_all.T),
                "srelb": np.ascontiguousarray(srelb_all.T),
                "cnt1": cnt1,
            }
        )
        slot_maps.append([(bounds[c] + sl[0], sl[1]) for sl in slots])
    w_arr = tuple(
        max(
            (sm[g][1] for sm in slot_maps if g < len(sm)),
            default=1,
        )
        for g in range(n_slots)
    )
    return t_l_arr, t_h_arr, m_l_arr, m_h_arr, w_arr, in_maps, slot_maps


def kernel(features, neighbor_idx, segment_ids):
    global LAST_NC
    t_l_arr, t_h_arr, m_l_arr, m_h_arr, w_arr, in_maps, slot_maps = _preprocess(
        features, neighbor_idx, segment_ids
    )

    key = (t_l_arr, t_h_arr, m_l_arr, m_h_arr, w_arr)
    if key not in _PROGRAM_CACHE:
        _PROGRAM_CACHE[key] = _build_program(
            t_l_arr, t_h_arr, m_l_arr, m_h_arr, w_arr
        )
    nc = _PROGRAM_CACHE[key]
    LAST_NC = nc

    try:
        res = bass_utils.run_bass_kernel_spmd(
            nc, in_maps, core_ids=list(range(N_CORES))
        )
    except Exception:
        # transient axon/device hiccups (e.g. recovering from a prior wedge)
        # have been observed to clear after a short pause
        import time

        time.sleep(20)
        res = bass_utils.run_bass_kernel_spmd(
            nc, in_maps, core_ids=list(range(N_CORES))
        )

    out = np.empty((N_NODES, D), np.float32)
    for c in range(N_CORES):
        oc = res.results[c]["out"]
        for g, (abs_base, width) in enumerate(slot_maps[c]):
            out[abs_base : abs_base + width] = oc[g * 128 : g * 128 + width].astype(
                np.float32
            )
    return out
